# revision 16
# baseline (speedup 1.0000x reference)
"""Trainium2 Bass kernel for nn_EncoderLayer_intent (MatNet-style encoder).

Contract: kernel(**inputs) takes FULL inputs (B=16 batches), shards batch
across 8 NeuronCores (2 per core), runs one compiled SPMD Bass program,
returns (ope_out, ma_out, veh_out) full-shape float32.
"""
import sys, os
sys.path.insert(0, '/opt/trn_rl_repo')
import numpy as np
import ml_dtypes

import concourse.bass as bass
import concourse.bacc as bacc
import concourse.tile as tile
from concourse import mybir
from concourse import bass_isa
from concourse.bass_utils import run_bass_kernel_spmd
from concourse.masks import make_identity

from concourse import dve_ops as DO
from concourse.dve_spec import (Spec, Src0, Src1, C0, C1, C2, Zero, relu,
                                maxx, lower, _has_src1)
from concourse.dve_uop import DveOpSpec

F32 = mybir.dt.float32
F32R = mybir.dt.float32r
BF16 = mybir.dt.bfloat16
U8 = mybir.dt.uint8
ALU = mybir.AluOpType
AF = mybir.ActivationFunctionType
AX = mybir.AxisListType

B, NO, NM, NV, E = 16, 512, 128, 64, 256
H, DK, MS, FF = 16, 16, 8, 512
NCORE = 8
BPC = B // NCORE  # batches per core
BIG = 1.0e9


# ---------------- custom DVE ops ----------------
def _register_op(name, spec):
    for op in DO.OPS:
        if op.name == name:
            return op
    opcode = DO._CUSTOM_DVE_ROW_BASE + len(DO.OPS)
    uops = lower(spec, ver="v3")
    s = DveOpSpec(name=name, opcode=opcode, uops=uops, rd1_en=_has_src1(spec))
    op = DO.DveOp(name, spec, subdim=False, uops_sha={"v3": s.sha("v3")})
    DO.OPS.append(op)
    DO.CUSTOM_DVE_SPECS[name] = spec
    DO._SUB_OPCODE_FOR_NAME[name] = opcode
    return op


# out = C0*x + C1*|x| + Src1
F0_ACC = _register_op("F0_ACC_ANT", Spec(
    body=Src0 * C0 + maxx(Src0, Zero - Src0) * C1 + Src1,
    reference=lambda in0, in1, s0, s1, imm2:
        (in0 * s0 + np.abs(in0) * s1 + in1).astype(np.float32)))
# out = C1*|x - C0| + C2   (chain seed)
KNOT_NEW = _register_op("KNOT_NEW_ANT", Spec(
    body=maxx(Src0 - C0, C0 - Src0) * C1 + C2,
    reference=lambda in0, in1, s0, s1, imm2:
        (np.abs(in0 - s0) * s1 + imm2).astype(np.float32)))
# out = Src1 + C1*|x - C0| + C2
KNOT_ACC = _register_op("KNOT_ACC_ANT", Spec(
    body=Src1 + maxx(Src0 - C0, C0 - Src0) * C1 + C2,
    reference=lambda in0, in1, s0, s1, imm2:
        (in1 + np.abs(in0 - s0) * s1 + imm2).astype(np.float32)))
# out = relu(C0*x + C1*y)
RELU_AFF2 = _register_op("RELU_AFF2_ANT", Spec(
    body=relu(Src0 * C0 + Src1 * C1),
    reference=lambda in0, in1, s0, s1, imm2:
        np.maximum(in0 * s0 + in1 * s1, 0.0).astype(np.float32)))


def _bcast_ap(t_ap, ap):
    return bass.AP(tensor=t_ap.tensor, offset=t_ap.offset, ap=ap)


def _f(x):
    return float(np.float32(x))


# ---------------- host precompute ----------------
def _host_coefs(mixW1, mixb1, mixW2, mixb2):
    """Per (block i, head h): a[s], b[s], w[s]; collapsed-PWL coefs."""
    assert np.all(mixb1 == 0.0), "fast path requires mixb1 == 0"
    co = {}
    a = mixW1[:, :, 0, :].astype(np.float64)   # [4, H, MS]
    bb = mixW1[:, :, 1, :].astype(np.float64)
    w = mixW2[:, :, :, 0].astype(np.float64)   # [4, H, MS]
    co['a'], co['b'], co['w'] = a, bb, w
    co['lam'] = 0.5 * (w * a).sum(-1)          # [4, H]
    co['kap'] = 0.5 * (w * np.abs(a)).sum(-1)  # [4, H]
    # ope delta-f: C + sum_s kp_s |x - t_s| - kap|x|
    az = (a == 0.0)
    safe_a = np.where(az, 1.0, a)
    co['t'] = np.where(az, 0.0, -bb / safe_a)          # [4, H, MS]
    co['kp'] = np.where(az, 0.0, 0.5 * w * np.abs(a))  # [4, H, MS]
    co['C'] = (0.5 * w * bb).sum(-1) + (np.where(az, 0.5 * w * np.abs(bb), 0.0)).sum(-1)
    return co


# ---------------- program builder ----------------
class Prog:
    pass


def build_program(host):
    """host: dict with precomputed coefficient arrays (all float64)."""
    co = host['co']
    nc = bacc.Bacc("TRN2", target_bir_lowering=False, debug=False,
                   num_devices=NCORE)
    P = Prog()
    P.nc = nc

    dt_in = {}

    def din(name, shape, dt=F32):
        dt_in[name] = (shape, dt)
        return nc.dram_tensor(name, list(shape), dt, kind="ExternalInput")

    # per-core data (BPC batches)
    ope_d = din("ope_emb", [BPC, NO, E])
    ma_d = din("ma_emb", [BPC, NM, E])
    veh_d = din("veh_emb", [BPC, NV, E])
    proc_d = din("proc_time", [BPC, NO, NM])
    trans_d = din("trans_time", [BPC, NM, NM])
    mv_d = din("MVpair_trans", [BPC, NV, NM])
    mdyn_d = din("mask_dyn", [BPC, NO, NM], U8)
    mma_f_d = din("mask_ma_f", [BPC, NM])       # float mask
    mma_nf_d = din("mask_ma_nf", [BPC, NM])     # negated float mask

    # weights (f32r, host passes float32 arrays)
    HDP = H * 2 * DK  # 512: heads padded to 32 rows
    wq_d = din("Wq", [4, E, HDP], BF16)   # 0.25 folded + head-padded on host
    wk_d = din("Wk", [4, E, HDP], BF16)
    wv_d = din("Wv", [4, E, HDP], BF16)
    wo_d = din("Wo", [4, HDP, E], F32R)
    w1_d = din("fW1", [4, E, FF], BF16)
    w2_d = din("fW2", [4, FF, E], BF16)
    mpw_d = din("maprojW", [2 * E, E], F32R)
    fb1_d = din("ffb1", [4, FF])
    fb2_d = din("ffb2", [4, E])
    mpb_d = din("maprojb", [E])
    cvec_d = din("cvec", [128, host['cvec'].shape[1]])  # packed [P,1] consts
    ibf_d = din("I_bf16", [128, 128], BF16)
    ifr_d = din("I_f32r", [128, 128], F32R)
    if32_d = din("I_f32", [128, 128], F32)

    # outputs
    opeo_d = nc.dram_tensor("ope_out", [BPC, NO, E], F32, kind="ExternalOutput")
    mao_d = nc.dram_tensor("ma_out", [BPC, NM, E], F32, kind="ExternalOutput")
    veho_d = nc.dram_tensor("veh_out", [BPC, NV, E], F32, kind="ExternalOutput")

    # internal dram bounce for per-row min values (ope)
    xt_d = [nc.dram_tensor(f"xt_bounce{b}", [NO], F32) for b in range(BPC)]

    P.input_specs = dt_in

    from contextlib import ExitStack
    with tile.TileContext(nc) as tc, ExitStack() as ctx:
        wsb = ctx.enter_context(tc.tile_pool(name="wsb", bufs=1))
        data = ctx.enter_context(tc.tile_pool(name="data", bufs=2))
        wk_pool = ctx.enter_context(tc.tile_pool(name="work", bufs=2))
        big = ctx.enter_context(tc.tile_pool(name="bigw", bufs=1))
        ps = ctx.enter_context(tc.tile_pool(name="ps", bufs=2, space="PSUM"))

        # ---- persistent constants ----
        I_bf = wsb.tile([128, 128], BF16, tag="I_bf")
        nc.sync.dma_start(out=I_bf[:], in_=ibf_d[:])
        I_fr = wsb.tile([128, 128], F32R, tag="I_fr")
        nc.sync.dma_start(out=I_fr[:], in_=ifr_d[:])
        I_f32 = wsb.tile([128, 128], F32, tag="I_f32")
        nc.sync.dma_start(out=I_f32[:], in_=if32_d[:])
        eps_t = wsb.tile([128, 1], F32, tag="eps")
        nc.vector.memset(eps_t[:], 1e-5)
        cvec = wsb.tile([128, host['cvec'].shape[1]], F32, tag="cvec")
        nc.sync.dma_start(out=cvec[:], in_=cvec_d[:])

        # weights into SBUF (per block chunked by 128 partitions)
        W = {}
        for nm, d, kchunks, width, wdt in (
                ("wq", wq_d, 2, 512, BF16), ("wk", wk_d, 2, 512, BF16),
                ("wv", wv_d, 2, 512, BF16), ("wo", wo_d, 4, 256, F32R),
                ("w1", w1_d, 2, FF, BF16), ("w2", w2_d, 4, 256, BF16)):
            for i in range(4):
                for k in range(kchunks):
                    t = wsb.tile([128, width], wdt, tag=f"{nm}{i}{k}")
                    nc.sync.dma_start(out=t[:], in_=d[i, k * 128:(k + 1) * 128, :])
                    W[(nm, i, k)] = t
        for j in range(4):
            t = wsb.tile([128, 256], F32R, tag=f"mpw{j}")
            nc.sync.dma_start(out=t[:], in_=mpw_d[j * 128:(j + 1) * 128, :])
            W[("mpw", j)] = t
        FB1, FB2B = {}, {}
        for i in range(4):
            t = wsb.tile([128, 4], F32, tag=f"fb1_{i}")
            nc.sync.dma_start(out=t[:], in_=fb1_d[i].rearrange("(f p) -> p f", p=128))
            FB1[i] = t
            t2 = wsb.tile([128, 256], F32, tag=f"fb2_{i}")
            a2 = fb2_d[i]
            nc.sync.dma_start(out=t2[:], in_=_bcast_ap(a2, [[0, 128], [1, 256]]))
            FB2B[i] = t2
        mpbB = wsb.tile([128, 256], F32, tag="mpb")
        nc.sync.dma_start(out=mpbB[:], in_=_bcast_ap(mpb_d[:], [[0, 128], [1, 256]]))

        # =========================================================
        def transpose_to(nc, pool, pspool, src_ap, rows, cols, out_dt, tag):
            """src_ap [rows<=128, cols] sbuf -> returns sbuf tile [cols<=?..]
            transposed by 128-col tiles. cols<=512, rows<=128."""
            out = pool.tile([128, rows], out_dt, tag=tag) if cols <= 128 else None
            assert cols <= 128
            pt = pspool.tile([128, rows], F32, tag="scr")
            ident = I_bf if src_ap.dtype == BF16 else I_fr
            nc.tensor.transpose(pt[0:cols, 0:rows], src_ap, ident[0:rows, 0:rows])
            nc.vector.tensor_copy(out[0:cols, 0:rows], pt[0:cols, 0:rows])
            return out

        def ln_rows(nc, pool, u_ap, rows, out_ap):
            """LayerNorm over free dim (256) of u_ap [rows,256] -> out_ap."""
            st = pool.tile([128, 6], F32, tag="ln_st")
            mv = pool.tile([128, 2], F32, tag="ln_mv")
            nc.vector.bn_stats(st[0:rows, :], u_ap)
            nc.vector.bn_aggr(mv[0:rows, :], st[0:rows, :])
            lnv = pool.tile([128, 1], F32, tag="ln_l")
            rstd = pool.tile([128, 1], F32, tag="ln_r")
            nc.scalar.activation(lnv[0:rows, :], mv[0:rows, 1:2], AF.Ln,
                                 bias=eps_t[0:rows, :], scale=1.0)
            nc.scalar.activation(rstd[0:rows, :], lnv[0:rows, :], AF.Exp,
                                 bias=0.0, scale=-0.5)
            nc.vector.tensor_scalar(out_ap, u_ap, mv[0:rows, 0:1], rstd[0:rows, 0:1],
                                    op0=ALU.subtract, op1=ALU.mult)

        # =========================================================
        for b in range(BPC):
            # ---------- loads ----------
            row_ope = data.tile([128, 4, E], F32, tag="row_ope")
            nc.sync.dma_start(out=row_ope[:],
                              in_=ope_d[b].rearrange("(t p) e -> p t e", p=128))
            row_ma = data.tile([128, E], F32, tag="row_ma")
            nc.sync.dma_start(out=row_ma[:], in_=ma_d[b])
            row_veh = data.tile([128, E], F32, tag="row_veh")
            nc.sync.dma_start(out=row_veh[0:NV, :], in_=veh_d[b])
            proc = data.tile([128, 4, NM], F32, tag="proc")
            nc.sync.dma_start(out=proc[:],
                              in_=proc_d[b].rearrange("(t p) c -> p t c", p=128))
            trans = data.tile([128, NM], F32, tag="trans")
            nc.sync.dma_start(out=trans[:], in_=trans_d[b])
            mv_t = data.tile([128, NM], F32, tag="mv")
            nc.sync.dma_start(out=mv_t[0:NV, :], in_=mv_d[b])
            mdyn = data.tile([128, 4, NM], U8, tag="mdyn")
            nc.sync.dma_start(out=mdyn[:],
                              in_=mdyn_d[b].rearrange("(t p) c -> p t c", p=128))
            mrow = data.tile([128, NM], F32, tag="mrow")  # mask_ma bcast on parts
            nc.sync.dma_start(out=mrow[:],
                              in_=_bcast_ap(mma_f_d[b], [[0, 128], [1, NM]]))
            mcol_n = data.tile([128, 1], F32, tag="mcol_n")  # -mask per partition
            nc.sync.dma_start(out=mcol_n[:],
                              in_=mma_nf_d[b].rearrange("(p one) -> p one", one=1))

            # ---------- adjacency ----------
            # ope: y (one-hot-ish), min values
            mdf = wk_pool.tile([128, 4, NM], F32, tag="mdf")
            nc.gpsimd.tensor_copy(mdf[:], mdyn[:])
            Ap = proc  # in-place: proc_time not needed afterwards
            nc.vector.tensor_tensor(Ap[:], proc[:], mdf[:], op=ALU.mult)
            z0 = mdf  # reuse
            nc.vector.tensor_scalar(z0[:], Ap[:], 0.0, None, op0=ALU.is_equal)
            nc.vector.scalar_tensor_tensor(z0[:], z0[:], BIG, Ap[:],
                                           op0=ALU.mult, op1=ALU.add)
            mn = wk_pool.tile([128, 4], F32, tag="mn")
            nc.vector.tensor_reduce(mn[:], z0[:], axis=AX.X, op=ALU.min)
            y_ope = wk_pool.tile([128, 4, NM], BF16, tag="y_ope")
            mn_b = _bcast_ap(mn[:], [[mn.ap[0][0], 128], [1, 4], [0, NM]])
            nc.vector.tensor_tensor(y_ope[:], Ap[:], mn_b, op=ALU.is_equal)
            # bounce min values to r-order and broadcast
            nc.sync.dma_start(out=xt_d[b][:].rearrange("(t p) -> p t", p=128),
                              in_=mn[:])
            xrep = wk_pool.tile([16, NO], F32, tag="xrep")
            nc.sync.dma_start(out=xrep[:],
                              in_=_bcast_ap(xt_d[b][:], [[0, 16], [1, NO]]))

            # ma1: y2 = 1 - m m' ; ma2: A_on
            mask2 = wk_pool.tile([128, NM], F32, tag="mask2")
            nc.vector.tensor_scalar(mask2[:], mrow[:], mcol_n[:, 0:1], None,
                                    op0=ALU.mult)  # -m m'
            y2 = wk_pool.tile([128, NM], F32, tag="y2")
            nc.vector.tensor_scalar(y2[:], mask2[:], 1.0, None, op0=ALU.add)
            ttm = wk_pool.tile([128, NM], F32, tag="ttm")
            # tt = (trans-101)*mask2f + 101 ; mask2f = -mask2... mask2 holds -m m'
            nc.vector.tensor_scalar(mask2[:], mask2[:], -1.0, None, op0=ALU.mult)
            nc.vector.scalar_tensor_tensor(ttm[:], trans[:], -101.0, mask2[:],
                                           op0=ALU.add, op1=ALU.mult)
            nc.vector.tensor_scalar(ttm[:], ttm[:], 101.0, None, op0=ALU.add)
            y_on = wk_pool.tile([128, NM], F32, tag="y_on")
            _inv_norm_tile(nc, wk_pool, ttm, 128, y_on)
            # veh: A_off
            # veh cost, replicated to both packed head-halves (rows 0:64, 64:128)
            yoffP = wk_pool.tile([128, NM], F32, tag="yoffP")
            _inv_norm_tile(nc, wk_pool, mv_t, NV, yoffP, rows=NV)
            nc.sync.dma_start(out=yoffP[64:128, :], in_=yoffP[0:64, :])

            # ---------- transposed embeddings ----------
            maT = []
            for k in range(2):
                t = wk_pool.tile([128, NM], BF16, tag=f"maT{k}")
                pt = ps.tile([128, NM], F32, tag="scr")
                nc.tensor.transpose(pt[:, :], row_ma[:, k * 128:(k + 1) * 128],
                                    I_f32[:, :])
                nc.vector.tensor_copy(t[:], pt[:])
                maT.append(t)
            vehT = []
            for k in range(2):
                t = wk_pool.tile([128, NV], BF16, tag=f"vehT{k}")
                pt = ps.tile([128, NV], F32, tag="scr")
                nc.tensor.transpose(pt[:, 0:NV],
                                    row_veh[0:NV, k * 128:(k + 1) * 128],
                                    I_f32[0:NV, 0:NV])
                nc.vector.tensor_copy(t[:, 0:NV], pt[:, 0:NV])
                vehT.append(t)
            opeT = []
            for k in range(2):
                t = wk_pool.tile([128, NO], BF16, tag=f"opeT{k}")
                for rt in range(4):
                    pt = ps.tile([128, 128], F32, tag="scr")
                    nc.tensor.transpose(pt[:, :],
                                        row_ope[:, rt, k * 128:(k + 1) * 128],
                                        I_f32[:, :])
                    nc.vector.tensor_copy(t[:, rt * 128:(rt + 1) * 128], pt[:])
                opeT.append(t)

            # =====================================================
            # Block runner
            # =====================================================
            def qkv(i, rT, R, C=NM):
                """qT_bf/kT_bf: 4 chunks [128,R] (heads padded to 32 rows);
                v_bf [128(C), 512] head-padded row-major."""
                outs = []
                for nm, srcT, n in (("wq", rT, R), ("wk", maT, C)):
                    chunks = []
                    for m in range(4):
                        pq = ps.tile([128, 512], F32, tag="scr")
                        for k in range(2):
                            nc.tensor.matmul(pq[:, 0:n],
                                             W[(nm, i, k)][:, m * 128:(m + 1) * 128],
                                             srcT[k][:, 0:n],
                                             start=(k == 0), stop=(k == 1))
                        sb_t = wk_pool.tile([128, n], BF16, tag=f"{nm}T{m}")
                        nc.scalar.copy(sb_t[:, 0:n], pq[:, 0:n])
                        chunks.append(sb_t)
                    outs.append(chunks)
                # v row-major (padded): lhsT = maT chunks, rhs = Wv'
                pv = ps.tile([128, 512], F32, tag="scr")
                for k in range(2):
                    nc.tensor.matmul(pv[:, 0:512], maT[k][:, 0:C], W[("wv", i, k)][:],
                                     start=(k == 0), stop=(k == 1))
                v_bf = wk_pool.tile([128, 512], BF16, tag="v_bf")
                nc.scalar.copy(v_bf[:], pv[:, 0:512])
                return outs[0], outs[1], v_bf

            def block_tail(i, ctxT_sb, row_ap, R, out_f32, rts, out_dma=None):
                """Wo + residual + LN + FFN + residual + LN -> out_f32 or DMA"""
                out1 = wk_pool.tile([128, rts, E], F32R, tag="out1")
                for rt in range(rts):
                    rows = min(128, R - rt * 128)
                    po = ps.tile([128, E], F32, tag="scr")
                    for m in range(4):
                        nc.tensor.matmul(po[0:rows, :],
                                         ctxT_sb[m][:, rt * 128:rt * 128 + rows],
                                         W[("wo", i, m)][:],
                                         start=(m == 0), stop=(m == 3))
                    u = wk_pool.tile([128, E], F32, tag="u_res")
                    nc.vector.tensor_tensor(u[0:rows, :], po[0:rows, :],
                                            row_ap[rt][0:rows, :], op=ALU.add)
                    ln_rows(nc, wk_pool, u[0:rows, :], rows,
                            out1[0:rows, rt, :])
                # out1T
                out1T = []
                for k in range(2):
                    t = wk_pool.tile([128, R], BF16, tag=f"out1T{k}")
                    for rt in range(rts):
                        rows = min(128, R - rt * 128)
                        pt = ps.tile([128, 128], F32R, tag="scr")
                        nc.tensor.transpose(pt[:, 0:rows],
                                            out1[0:rows, rt, k * 128:(k + 1) * 128],
                                            I_fr[0:rows, 0:rows])
                        nc.vector.tensor_copy(t[:, rt * 128:rt * 128 + rows],
                                              pt[:, 0:rows])
                    out1T.append(t)
                # ffn: h1T chunks to sbuf, then per-rt out2 accumulation
                h1s = []
                for f in range(4):
                    ph = ps.tile([128, 512], F32, tag="scr")
                    for k in range(2):
                        nc.tensor.matmul(ph[:, 0:R],
                                         W[("w1", i, k)][:, f * 128:(f + 1) * 128],
                                         out1T[k][:, 0:R],
                                         start=(k == 0), stop=(k == 1))
                    h1 = wk_pool.tile([128, R], BF16, tag=f"h1T{f}",
                                      name=f"h1T{f}")
                    nc.scalar.activation(h1[:, 0:R], ph[:, 0:R], AF.Relu,
                                         bias=FB1[i][:, f:f + 1], scale=1.0)
                    h1s.append(h1)
                for rt in range(rts):
                    rows = min(128, R - rt * 128)
                    out2 = ps.tile([128, E], F32, tag="scr", name="out2ps")
                    for f in range(4):
                        nc.tensor.matmul(out2[0:rows, :],
                                         h1s[f][:, rt * 128:rt * 128 + rows],
                                         W[("w2", i, f)][:],
                                         start=(f == 0), stop=(f == 3))
                    u = wk_pool.tile([128, E], F32, tag="u_res")
                    nc.vector.tensor_tensor(u[0:rows, :], out2[0:rows, :],
                                            out1[0:rows, rt, :], op=ALU.add)
                    nc.vector.tensor_tensor(u[0:rows, :], u[0:rows, :],
                                            FB2B[i][0:rows, :], op=ALU.add)
                    if out_f32 is not None:
                        ln_rows(nc, wk_pool, u[0:rows, :], rows,
                                out_f32[0:rows, rt, :])
                    else:
                        fo = wk_pool.tile([128, E], F32, tag="finout")
                        ln_rows(nc, wk_pool, u[0:rows, :], rows, fo[0:rows, :])
                        nc.sync.dma_start(out=out_dma(rt), in_=fo[0:rows, :])

            def attn_pipeline(i, qT_bf, kT_bf, v_bf, R, rts, score_fn, packed_veh=False):
                """dots per head -> score_fn fills SC bf16; softmax; attnT; ctxT"""
                nheads = H
                SC = big.tile([128, nheads, rts, 128], BF16, tag="SC")
                for h in range(nheads):
                    dp = ps.tile([128, rts, 128], F32, tag="dot")
                    ch, off = h // 4, (h % 4) * 32
                    for rt in range(rts):
                        rows = min(128, R - rt * 128)
                        nc.tensor.matmul(dp[0:rows, rt, :],
                                         qT_bf[ch][off:off + 16,
                                                   rt * 128:rt * 128 + rows],
                                         kT_bf[ch][off:off + 16, 0:NM],
                                         start=True, stop=True,
                                         tile_position=(off, 0))
                    score_fn(h, dp, SC)
                # softmax over c (free innermost)
                den = wk_pool.tile([128, nheads * rts], F32, tag="den")
                scf = SC[:].rearrange("p h t c -> p (h t) c")
                nc.scalar.activation(SC[:].rearrange("p h t c -> p (h t c)"),
                                     SC[:].rearrange("p h t c -> p (h t c)"),
                                     AF.Exp, bias=0.0, scale=1.0)
                nc.vector.tensor_reduce(den[:], scf, axis=AX.X, op=ALU.add)
                nc.vector.reciprocal(den[:], den[:])
                denb = wk_pool.tile([128, nheads * rts], BF16, tag="denb")
                nc.vector.tensor_copy(denb[:], den[:])
                nc.vector.tensor_tensor(
                    scf, scf,
                    _bcast_ap(denb[:], [[denb.ap[0][0], 128], [1, nheads * rts], [0, 128]]),
                    op=ALU.mult)
                # transpose attn + ctxT (v stationary, attnT moving)
                ctxT_ps = [ps.tile([128, R], F32, tag="ctx", bufs=4, name=f"ctxps{_i}") for _i in range(4)]
                for h in range(nheads):
                    ch, off = h // 4, (h % 4) * 32
                    at = ps.tile([128, rts * 128], BF16, tag="scr")
                    for rt in range(rts):
                        rows = min(128, R - rt * 128)
                        nc.tensor.transpose(at[:, rt * 128:rt * 128 + rows],
                                            SC[0:rows, h, rt, :],
                                            I_bf[0:rows, 0:rows])
                    atb = wk_pool.tile([128, rts * 128], BF16, tag="atb")
                    nc.vector.tensor_copy(atb[:], at[:])
                    nc.tensor.matmul(ctxT_ps[ch][off:off + 32, 0:R],
                                     v_bf[:, h * 32:(h + 1) * 32],
                                     atb[:, 0:R], start=True, stop=True,
                                     tile_position=(0, off))
                ctxT_sb = []
                for m in range(4):
                    t = wk_pool.tile([128, R], F32R, tag=f"ctxT{m}")
                    nc.vector.tensor_copy(t[:, 0:R], ctxT_ps[m][:, 0:R])
                    ctxT_sb.append(t)
                return ctxT_sb

            # ----- ope block (i=0) -----
            i = 0
            qT_bf, kT_bf, v_bf = qkv(0, opeT, NO)
            # delta-f per (h, r): chain on [16, 512]
            dd = wk_pool.tile([16, NO], F32, tag="dd")
            col0 = host['cvec_cols']['ope_t0']
            for s in range(MS + 1):
                tcol = cvec[0:16, col0 + 2 * s: col0 + 2 * s + 1]
                kcol = cvec[0:16, col0 + 2 * s + 1: col0 + 2 * s + 2]
                if s == 0:
                    nc.vector._custom_dve(KNOT_NEW, out=dd[:], in0=xrep[:],
                                          s0=tcol, s1=kcol)
                else:
                    nc.vector._custom_dve(KNOT_ACC, out=dd[:], in0=xrep[:],
                                          in1=dd[:], s0=tcol, s1=kcol)
            ccol = host['cvec_cols']['ope_C']
            nc.vector.tensor_scalar(dd[:], dd[:], cvec[0:16, ccol:ccol + 1], None,
                                    op0=ALU.add)
            # transpose dd -> D [128 r, 4 rt, 16 h]
            ddb = wk_pool.tile([16, NO], BF16, tag="ddb")
            nc.vector.tensor_copy(ddb[:], dd[:])
            D = wk_pool.tile([128, 4, 16], BF16, tag="D")
            for rt in range(4):
                pt = ps.tile([128, 16], BF16, tag="scr")
                nc.tensor.transpose(pt[:, 0:16], ddb[:, rt * 128:(rt + 1) * 128],
                                    I_bf[0:16, 0:16])
                nc.vector.tensor_copy(D[:, rt, :], pt[:, 0:16])

            def ope_score(h, dp, SC):
                yD = wk_pool.tile([128, 4, 128], BF16, tag="yD")
                d_h = bass.AP(tensor=D.tensor, offset=D.offset + h,
                              ap=[[D.ap[0][0], 128], [16, 4], [0, 128]])
                nc.vector.tensor_tensor(yD[:], y_ope[:], d_h, op=ALU.mult)
                nc.vector._custom_dve(
                    F0_ACC, out=SC[:, h, :, :], in0=dp[:],
                    in1=yD[:], s0=_f(co['lam'][0, h]), s1=_f(co['kap'][0, h]))

            ctxT_sb = attn_pipeline(0, qT_bf, kT_bf, v_bf, NO, 4, ope_score)
            block_tail(0, ctxT_sb, [row_ope[:, rt, :] for rt in range(4)],
                       NO, None, 4,
                       out_dma=lambda rt: opeo_d[b].rearrange(
                           "(t p) e -> p t e", p=128)[:, rt, :])

            # ----- veh block (i=1), heads packed 2-per-tile -----
            i = 1
            qT_bf, kT_bf, v_bf = qkv(1, vehT, NV)
            SCV = big.tile([128, 8, 128], BF16, tag="SCV")
            for pair in range(8):
                dp = ps.tile([128, 128], F32, tag="dot")
                for half in range(2):
                    h = pair * 2 + half
                    ch, off = h // 4, (h % 4) * 32
                    nc.tensor.matmul(dp[half * 64:half * 64 + 64, :],
                                     qT_bf[ch][off:off + 16, 0:NV],
                                     kT_bf[ch][off:off + 16, 0:NM],
                                     start=True, stop=True,
                                     tile_position=(off, half * 64))
                # score: sum_s w relu(a x + b y_off)
                acc = wk_pool.tile([128, 128], F32, tag="accv")
                u = wk_pool.tile([128, 128], F32, tag="uv")
                cb = host['cvec_cols']['veh'] + pair * (3 * MS)
                for s in range(MS):
                    nc.vector._custom_dve(
                        RELU_AFF2, out=u[:], in0=dp[:], in1=yoffP[:],
                        s0=cvec[:, cb + 3 * s:cb + 3 * s + 1],
                        s1=cvec[:, cb + 3 * s + 1:cb + 3 * s + 2])
                    if s == 0:
                        nc.vector.tensor_scalar(
                            acc[:], u[:], cvec[:, cb + 2:cb + 3], None, op0=ALU.mult)
                    else:
                        nc.vector.scalar_tensor_tensor(
                            acc[:], u[:], cvec[:, cb + 3 * s + 2:cb + 3 * s + 3],
                            acc[:], op0=ALU.mult, op1=ALU.add)
                nc.vector.tensor_copy(SCV[:, pair, :], acc[:])
            # softmax veh
            denv = wk_pool.tile([128, 8], F32, tag="denv")
            nc.scalar.activation(SCV[:].rearrange("p a c -> p (a c)"),
                                 SCV[:].rearrange("p a c -> p (a c)"),
                                 AF.Exp, bias=0.0, scale=1.0)
            nc.vector.tensor_reduce(denv[:], SCV[:], axis=AX.X, op=ALU.add)
            nc.vector.reciprocal(denv[:], denv[:])
            denvb = wk_pool.tile([128, 8], BF16, tag="denvb")
            nc.vector.tensor_copy(denvb[:], denv[:])
            nc.vector.tensor_tensor(
                SCV[:], SCV[:],
                _bcast_ap(denvb[:], [[denvb.ap[0][0], 128], [1, 8], [0, 128]]),
                op=ALU.mult)
            ctxT_ps = [ps.tile([128, NV], F32, tag="ctx", bufs=4, name=f"ctxpsv{_i}") for _i in range(4)]
            for pair in range(8):
                at = ps.tile([128, 128], BF16, tag="scr")
                nc.tensor.transpose(at[:], SCV[:, pair, :], I_bf[:, :])
                atb = wk_pool.tile([128, 128], BF16, tag="atb")
                nc.vector.tensor_copy(atb[:], at[:])
                for half in range(2):
                    h = pair * 2 + half
                    ch, off = h // 4, (h % 4) * 32
                    nc.tensor.matmul(ctxT_ps[ch][off:off + 32, 0:NV],
                                     v_bf[:, h * 32:(h + 1) * 32],
                                     atb[:, half * 64:half * 64 + 64],
                                     start=True, stop=True,
                                     tile_position=(0, off))
            ctxT_sb = []
            for m in range(4):
                t = wk_pool.tile([128, NV], F32R, tag=f"ctxT{m}")
                nc.vector.tensor_copy(t[:, 0:NV], ctxT_ps[m][:, 0:NV])
                ctxT_sb.append(t)
            block_tail(1, ctxT_sb, [row_veh], NV, None, 1,
                       out_dma=lambda rt: veho_d[b])

            # ----- ma1 block (i=2) -----
            i = 2
            qT_bf, kT_bf, v_bf = qkv(2, maT, NM)

            def ma1_score(h, dp, SC):
                dch = wk_pool.tile([128, 128], F32, tag="dch")
                for s in range(MS):
                    tv, kv = _f(co['t'][2, h, s]), _f(co['kp'][2, h, s])
                    if s == 0:
                        nc.vector._custom_dve(KNOT_NEW, out=dch[:], in0=dp[:],
                                              s0=tv, s1=kv, imm2=_f(co['C'][2, h]))
                    else:
                        nc.vector._custom_dve(KNOT_ACC, out=dch[:], in0=dp[:],
                                              in1=dch[:], s0=tv, s1=kv)
                nc.vector._custom_dve(KNOT_ACC, out=dch[:], in0=dp[:], in1=dch[:],
                                      s0=0.0, s1=_f(-co['kap'][2, h]))
                yd = wk_pool.tile([128, 128], F32, tag="yd")
                nc.vector.tensor_tensor(yd[:], y2[:], dch[:], op=ALU.mult)
                nc.vector._custom_dve(F0_ACC, out=SC[:, h, 0, :], in0=dp[:],
                                      in1=yd[:], s0=_f(co['lam'][2, h]),
                                      s1=_f(co['kap'][2, h]))

            ctxT_sb = attn_pipeline(2, qT_bf, kT_bf, v_bf, NM, 1, ma1_score)
            out_ma1 = wk_pool.tile([128, 1, E], F32, tag="out_ma1")
            block_tail(2, ctxT_sb, [row_ma], NM, out_ma1, 1)

            # ----- ma2 block (i=3) -----
            i = 3
            qT_bf, kT_bf, v_bf = qkv(3, maT, NM)

            def ma2_score(h, dp, SC):
                acc = wk_pool.tile([128, 128], F32, tag="accv")
                u = wk_pool.tile([128, 128], F32, tag="uv")
                for s in range(MS):
                    nc.vector._custom_dve(RELU_AFF2, out=u[:], in0=dp[:],
                                          in1=y_on[:], s0=_f(co['a'][3, h, s]),
                                          s1=_f(co['b'][3, h, s]))
                    if s == 0:
                        nc.vector.tensor_scalar(acc[:], u[:], _f(co['w'][3, h, s]),
                                                None, op0=ALU.mult)
                    elif s == MS - 1:
                        nc.vector.scalar_tensor_tensor(
                            SC[:, h, 0, :], u[:], _f(co['w'][3, h, s]), acc[:],
                            op0=ALU.mult, op1=ALU.add)
                    else:
                        nc.vector.scalar_tensor_tensor(
                            acc[:], u[:], _f(co['w'][3, h, s]), acc[:],
                            op0=ALU.mult, op1=ALU.add)

            ctxT_sb = attn_pipeline(3, qT_bf, kT_bf, v_bf, NM, 1, ma2_score)
            out_ma2 = wk_pool.tile([128, 1, E], F32, tag="out_ma2")
            block_tail(3, ctxT_sb, [row_ma], NM, out_ma2, 1)

            # ----- ma projection -----
            conT = []
            for j, (src, k) in enumerate([(out_ma1, 0), (out_ma1, 1),
                                          (out_ma2, 0), (out_ma2, 1)]):
                t = wk_pool.tile([128, NM], F32R, tag=f"conT{j}")
                pt = ps.tile([128, NM], F32, tag="scr")
                nc.tensor.transpose(pt[:], src[:, 0, k * 128:(k + 1) * 128],
                                    I_f32[:, :])
                nc.vector.tensor_copy(t[:], pt[:])
                conT.append(t)
            pm = ps.tile([128, E], F32, tag="scr")
            for j in range(4):
                nc.tensor.matmul(pm[:], conT[j][:], W[("mpw", j)][:],
                                 start=(j == 0), stop=(j == 3))
            out_ma = wk_pool.tile([128, E], F32, tag="out_ma")
            nc.vector.tensor_tensor(out_ma[:], pm[:], mpbB[:], op=ALU.add)
            nc.sync.dma_start(out=mao_d[b], in_=out_ma[:])

    nc.compile()
    return P


def _inv_norm_tile(nc, pool, src, chan, out, rows=128):
    """out = 1 - (src - mn)/(mx - mn) over whole [rows, NM] tile."""
    rmin = pool.tile([128, 1], F32, tag="inv_rmin")
    rmax = pool.tile([128, 1], F32, tag="inv_rmax")
    nc.vector.tensor_reduce(rmin[0:rows, :], src[0:rows, :], axis=AX.X, op=ALU.min)
    nc.vector.tensor_reduce(rmax[0:rows, :], src[0:rows, :], axis=AX.X, op=ALU.max)
    nc.vector.tensor_scalar(rmin[0:rows, :], rmin[0:rows, :], -1.0, None,
                            op0=ALU.mult)
    gmaxn = pool.tile([128, 1], F32, tag="inv_gmx")
    gminn = pool.tile([128, 1], F32, tag="inv_gmn")
    nc.gpsimd.partition_all_reduce(gmaxn[0:rows, :], rmax[0:rows, :],
                                   channels=rows, reduce_op=bass_isa.ReduceOp.max)
    nc.gpsimd.partition_all_reduce(gminn[0:rows, :], rmin[0:rows, :],
                                   channels=rows, reduce_op=bass_isa.ReduceOp.max)
    rng = pool.tile([128, 1], F32, tag="inv_rng")
    nc.vector.tensor_tensor(rng[0:rows, :], gmaxn[0:rows, :], gminn[0:rows, :],
                            op=ALU.add)  # mx - mn
    nc.vector.reciprocal(rng[0:rows, :], rng[0:rows, :])
    nc.vector.tensor_scalar(rng[0:rows, :], rng[0:rows, :], -1.0, None,
                            op0=ALU.mult)  # -1/(mx-mn)
    # out = (src - mx) * (-1/(mx-mn))
    nc.vector.tensor_scalar(out[0:rows, :], src[0:rows, :], gmaxn[0:rows, 0:1],
                            rng[0:rows, 0:1], op0=ALU.subtract, op1=ALU.mult)


# ---------------- host-side driver ----------------
_CACHE = {}


def kernel(**inputs):
    key = "prog"
    if key not in _CACHE:
        _CACHE[key] = _build_all(inputs)
    P, host = _CACHE[key]
    return _run(P, host, inputs)


def _build_all(inputs):
    co = _host_coefs(np.asarray(inputs['mixW1']), np.asarray(inputs['mixb1']),
                     np.asarray(inputs['mixW2']), np.asarray(inputs['mixb2']))
    # cvec packing
    cols = {}
    ncol = 0
    cv = np.zeros((128, 256), np.float64)
    # ope delta chain consts on partitions (h,s)->p = h*8+s  [16 rows used? no:
    # chain runs on [16, NO] with per-partition h] -> t/k per (h, s-col)
    cols['ope_t0'] = ncol
    for s in range(MS + 1):
        for h in range(H):
            if s < MS:
                cv[h, ncol + 2 * s] = co['t'][0, h, s]
                cv[h, ncol + 2 * s + 1] = co['kp'][0, h, s]
            else:
                cv[h, ncol + 2 * s] = 0.0
                cv[h, ncol + 2 * s + 1] = -co['kap'][0, h]
    ncol += 2 * (MS + 1)
    cols['ope_C'] = ncol
    for h in range(H):
        cv[h, ncol] = co['C'][0, h]
    ncol += 1
    # veh packed pair consts: for pair, s: a,b,w on partitions (half*64+r)
    cols['veh'] = ncol
    for pair in range(8):
        for s in range(MS):
            for half in range(2):
                h = pair * 2 + half
                rowsl = slice(half * 64, half * 64 + 64)
                cv[rowsl, ncol + pair * 3 * MS + 3 * s + 0] = co['a'][1, h, s]
                cv[rowsl, ncol + pair * 3 * MS + 3 * s + 1] = co['b'][1, h, s]
                cv[rowsl, ncol + pair * 3 * MS + 3 * s + 2] = co['w'][1, h, s]
    ncol += 8 * 3 * MS
    host = {'co': co, 'cvec': cv[:, 0:ncol].astype(np.float32),
            'cvec_cols': cols}
    P = build_program(host)
    return P, host


def _run(P, host, inputs):
    co = host['co']
    f32 = np.float32
    def padheads_cols(w):  # [4, E, H*DK] -> [4, E, H*2*DK], head cols at 32-blocks
        w = np.asarray(w, f32).reshape(4, E, H, DK)
        out = np.zeros((4, E, H, 2 * DK), f32)
        out[:, :, :, 0:DK] = w
        return out.reshape(4, E, H * 2 * DK)

    def padheads_rows(w):  # [4, H*DK, E] -> [4, H*2*DK, E]
        w = np.asarray(w, f32).reshape(4, H, DK, E)
        out = np.zeros((4, H, 2 * DK, E), f32)
        out[:, :, 0:DK, :] = w
        return out.reshape(4, H * 2 * DK, E)

    bf = ml_dtypes.bfloat16
    Wq = padheads_cols(np.asarray(inputs['Wq'], f32) * f32(1.0 / np.sqrt(DK)))
    in_common = {
        'Wq': Wq.astype(bf), 'Wk': padheads_cols(inputs['Wk']).astype(bf),
        'Wv': padheads_cols(inputs['Wv']).astype(bf),
        'Wo': padheads_rows(inputs['Wo']),
        'fW1': np.ascontiguousarray(inputs['ffW1'], f32).astype(bf),
        'fW2': np.ascontiguousarray(inputs['ffW2'], f32).astype(bf),
        'maprojW': np.ascontiguousarray(inputs['maprojW'], f32),
        'ffb1': np.ascontiguousarray(inputs['ffb1'], f32),
        'ffb2': np.ascontiguousarray(inputs['ffb2'], f32),
        'maprojb': np.ascontiguousarray(inputs['maprojb'], f32),
        'cvec': host['cvec'],
        'I_bf16': np.eye(128, dtype=f32).astype(bf),
        'I_f32r': np.eye(128, dtype=f32),
        'I_f32': np.eye(128, dtype=f32),
    }
    maps = []
    for c in range(NCORE):
        sl = slice(c * BPC, (c + 1) * BPC)
        mm = np.ascontiguousarray(inputs['mask_ma'][sl]).astype(np.float32)
        m = dict(in_common)
        m.update({
            'ope_emb': np.ascontiguousarray(inputs['ope_emb'][sl], f32),
            'ma_emb': np.ascontiguousarray(inputs['ma_emb'][sl], f32),
            'veh_emb': np.ascontiguousarray(inputs['veh_emb'][sl], f32),
            'proc_time': np.ascontiguousarray(inputs['proc_time'][sl], f32),
            'trans_time': np.ascontiguousarray(inputs['trans_time'][sl], f32),
            'MVpair_trans': np.ascontiguousarray(inputs['MVpair_trans'][sl], f32),
            'mask_dyn': np.ascontiguousarray(
                inputs['mask_dyn_ope_ma_adj'][sl]).astype(np.uint8),
            'mask_ma_f': mm, 'mask_ma_nf': -mm,
        })
        maps.append(m)
    trace = bool(int(os.environ.get('BASS_KERNEL_TRACE', '0')))
    res = run_bass_kernel_spmd(P.nc, maps, core_ids=list(range(NCORE)),
                               trace=trace)
    if trace:
        print('HW exec time:', res.exec_time_ns, 'ns')
        _CACHE['profile'] = res
    ope = np.concatenate([res.results[c]['ope_out'] for c in range(NCORE)], 0)
    mao = np.concatenate([res.results[c]['ma_out'] for c in range(NCORE)], 0)
    veh = np.concatenate([res.results[c]['veh_out'] for c in range(NCORE)], 0)
    return ope, mao, veh
